# revision 1
# baseline (speedup 1.0000x reference)
"""DGI (Deep Graph Infomax) forward kernel for 8 TRN2 NeuronCores.

Problem (all shapes hardcoded):
  seq1, seq2: [1, 8192, 128] f32   node features
  adj:        [1, 8192, 8192] f32  dense adjacency
  cc_label:   [8, 1024] i32        community partition (arange layout)
  W: [128,128], b: [128], Wb: [128,128], bb: [] f32
  out:        [1, 16384] f32       = concat(ret1, ret2)

Math per GCN branch: h = relu(adj @ (seq @ W) + b). We reassociate to
(adj @ seq) @ W so the big contraction uses natural-layout seq tiles as
the stationary operand and a host-transposed adj block as the moving
operand; everything then lives in "transposed" space (features on
partitions), where the community mean is a free-axis reduction and the
bilinear scores are a 1-column matmul.

Sharding: core k owns nodes [1024k, 1024k+1024) == community k (cc_label
is arange). Each core reads its adjT column block (32 MB), the full seqs
(8 MB, replicated), computes its 1024 scores per branch. No collectives.

Per-core device program (big matmuls in fp16: adj is pre-scaled by 256 on
the host to sit in fp16's normal range; the scale is undone for free in the
relu activation's `scale`; everything downstream is fp32):
  ZT[d, n]   = sum_m seq_s[m, d] * adjT[m, n]   (fp16, 64 accumulating
                                                 matmuls per psum bank,
                                                 split into two m-halves so
                                                 half 1's epilogue overlaps
                                                 half 2's stream)
  aggT[h, n] = sum_d W[d, h] * ZT[d, n]         (fp32)
  hT         = relu(aggT/256 + b)   (+ free-axis accum -> community sum)
  c          = sigmoid(sum / 1024)               [128, 1]
  cw         = Wb @ c     (lhsT = Wb^T from host) [128, 1]
  sc_s[n]    = sum_h hT_s[h, n] * cw[h] + bb     [1, 1024] per branch

Layouts are host-prepared so every DMA is partition-major with >=2KB
contiguous per-partition runs: adjt[p, t, n] = adj[node n of this core's
block, 128*t + p] * 256 (fp16), seq[p, t, d] = seq[128*t + p, d] (fp16).
adjacency streams on the sync HWDGE queue, seqs on the scalar queue, params
on gpsimd, so none of them serialize behind each other.
"""

import numpy as np

import concourse.bass as bass
import concourse.tile as tile
from concourse import bacc, mybir
from concourse.bass_utils import run_bass_kernel_spmd

N = 8192          # nodes
D = 128           # input feature dim
H = 128           # hidden dim
NC = 8            # communities / cores
CS = N // NC      # community size (nodes per core)
MT = N // 128     # number of 128-row m-tiles (64)
CHUNK = 512       # matmul moving free dim (psum bank width in fp32)
NCH = CS // CHUNK # n-chunks per core (2)

F32 = mybir.dt.float32
F16 = mybir.dt.float16
ADJ_SCALE = 256.0  # keeps fp16(adj*scale) in the normal range; undone in the relu


def _build_module() -> bass.Bass:
    nc = bacc.Bacc()

    adjt = nc.declare_dram_parameter("adjt", [128, MT, CS], F16, isOutput=False)
    seq1 = nc.declare_dram_parameter("seq1", [128, MT, D], F16, isOutput=False)
    seq2 = nc.declare_dram_parameter("seq2", [128, MT, D], F16, isOutput=False)
    w = nc.declare_dram_parameter("w", [D, H], F32, isOutput=False)
    wbt = nc.declare_dram_parameter("wbt", [H, H], F32, isOutput=False)
    bvec = nc.declare_dram_parameter("bvec", [H, 1], F32, isOutput=False)
    bbvec = nc.declare_dram_parameter("bbvec", [1, 1], F32, isOutput=False)
    out = nc.declare_dram_parameter("out", [2, CS], F32, isOutput=True)

    with tile.TileContext(nc) as tc:
        _emit(tc, adjt, seq1, seq2, w, wbt, bvec, bbvec, out)
    nc.finalize()
    return nc


def _emit(tc, adjt, seq1, seq2, w, wbt, bvec, bbvec, out):
    nc = tc.nc
    with (
        tc.tile_pool(name="singles", bufs=1) as singles,
        tc.tile_pool(name="adj_pool", bufs=3) as adj_pool,
        tc.tile_pool(name="misc", bufs=1) as misc,
        tc.tile_pool(name="psum", bufs=1, space="PSUM") as psum,
    ):
        w_sb = singles.tile([D, H], F32)
        nc.gpsimd.dma_start(out=w_sb, in_=w[:])
        wbt_sb = singles.tile([H, H], F32)
        nc.gpsimd.dma_start(out=wbt_sb, in_=wbt[:])
        b_sb = singles.tile([H, 1], F32)
        nc.gpsimd.dma_start(out=b_sb, in_=bvec[:])
        bb_sb = singles.tile([1, 1], F32)
        nc.gpsimd.dma_start(out=bb_sb, in_=bbvec[:])

        # Warmup adjacency group issued first on the scalar queue so the PE
        # can start while sync's first big group is in flight.
        WARM = 2
        adj_warm = adj_pool.tile([128, WARM, CS], F16, name="adj_warm", bufs=1)
        nc.scalar.dma_start(out=adj_warm, in_=adjt[:, 0:WARM, :])

        # Seqs staged in growing chunks so the first matmul isn't gated on
        # the full 4 MB.
        seq1_sb = singles.tile([128, MT, D], F16)
        seq2_sb = singles.tile([128, MT, D], F16)
        SEQ_CHUNKS = [4, 4, 8, 16, 16, 16]
        pos = 0
        for n in SEQ_CHUNKS:
            sl = slice(pos, pos + n)
            nc.scalar.dma_start(out=seq1_sb[:, sl, :], in_=seq1[:, sl, :])
            nc.scalar.dma_start(out=seq2_sb[:, sl, :], in_=seq2[:, sl, :])
            pos += n
        seq_sb = (seq1_sb, seq2_sb)

        HALF = MT // 2
        # Z accumulators split by m-half: first half banks 0-3, second 4-7.
        z_half = [
            [
                [psum.tile([128, CHUNK], F32, name=f"z_ps_{h}_{s}_{c}") for c in range(NCH)]
                for s in range(2)
            ]
            for h in range(2)
        ]
        zt_sb = [
            [
                [misc.tile([128, CHUNK], F32, name=f"zt_sb_{h}_{s}_{c}") for c in range(NCH)]
                for s in range(2)
            ]
            for h in range(2)
        ]
        h_sb = [
            [misc.tile([128, CHUNK], F32, name=f"h_sb_{s}_{c}") for c in range(NCH)]
            for s in range(2)
        ]
        csum = [misc.tile([H, 1], F32, name=f"csum_{c}") for c in range(NCH)]

        # (queue, n_tiles): tiny warmup group on the scalar queue lets the
        # PE start while sync's first big group is still in flight.
        ADJ_GROUPS = [("w", WARM), ("s", 6)] + [("s", 8)] * 7
        assert sum(n for _, n in ADJ_GROUPS) == MT

        def copy_ps(dst, src_ps, c):
            if c == 0:
                nc.vector.tensor_copy(out=dst, in_=src_ps)
            else:
                nc.scalar.activation(
                    out=dst, in_=src_ps, func=mybir.ActivationFunctionType.Copy
                )

        def emit_half1_copies():
            for s in range(2):
                for c in range(NCH):
                    copy_ps(zt_sb[0][s][c], z_half[0][s][c], c)

        def emit_half1_agg():
            # First-pass W-contraction into the (now free) first-half banks.
            for s in range(2):
                for c in range(NCH):
                    nc.tensor.matmul(
                        z_half[0][s][c], w_sb, zt_sb[0][s][c], start=True, stop=False
                    )

        t0 = 0
        for gi, (q, gn) in enumerate(ADJ_GROUPS):
            if q == "w":
                adj_sb = adj_warm
            else:
                adj_sb = adj_pool.tile([128, gn, CS], F16, name="adj_sb", tag="adj_sb", bufs=4)
                nc.sync.dma_start(out=adj_sb, in_=adjt[:, t0 : t0 + gn, :])
            for u in range(gn):
                t = t0 + u
                h = 0 if t < HALF else 1
                for s in range(2):
                    lhsT = seq_sb[s][:, t, :]
                    for c in range(NCH):
                        nc.tensor.matmul(
                            z_half[h][s][c],
                            lhsT,
                            adj_sb[:, u, c * CHUNK : (c + 1) * CHUNK],
                            start=(t % HALF == 0),
                            stop=(t % HALF == HALF - 1),
                        )
            t0 += gn
            if t0 - gn < HALF <= t0:
                emit_half1_copies()
            if t0 - gn < HALF + 16 <= t0:
                emit_half1_agg()

        # Tail: branch 0 (drives the sigmoid/cw chain) first; branch 1's
        # matmuls/relu fill the PE while scalar runs sigmoid.
        for c in range(NCH):
            copy_ps(zt_sb[1][0][c], z_half[1][0][c], c)
        for c in range(NCH):
            nc.tensor.matmul(
                z_half[0][0][c], w_sb, zt_sb[1][0][c], start=False, stop=True
            )
            nc.scalar.activation(
                out=h_sb[0][c],
                in_=z_half[0][0][c],
                func=mybir.ActivationFunctionType.Relu,
                bias=b_sb,
                scale=1.0 / ADJ_SCALE,
                accum_out=csum[c],
            )
        for c in range(NCH):
            copy_ps(zt_sb[1][1][c], z_half[1][1][c], c)

        csum_tot = misc.tile([H, 1], F32)
        nc.vector.tensor_add(out=csum_tot, in0=csum[0], in1=csum[1])
        c_sb = misc.tile([H, 1], F32)
        nc.scalar.activation(
            out=c_sb,
            in_=csum_tot,
            func=mybir.ActivationFunctionType.Sigmoid,
            scale=1.0 / CS,
        )

        for c in range(NCH):
            nc.tensor.matmul(
                z_half[0][1][c], w_sb, zt_sb[1][1][c], start=False, stop=True
            )
        cw_ps = z_half[1][0][0]
        nc.tensor.matmul(cw_ps[:, :1], wbt_sb, c_sb, start=True, stop=True)
        for c in range(NCH):
            nc.scalar.activation(
                out=h_sb[1][c],
                in_=z_half[0][1][c],
                func=mybir.ActivationFunctionType.Relu,
                bias=b_sb,
                scale=1.0 / ADJ_SCALE,
            )
        cw_sb = misc.tile([H, 1], F32)
        nc.vector.tensor_copy(out=cw_sb, in_=cw_ps[:, :1])

        out_sb = misc.tile([1, 2, CS], F32)
        sc_banks = [
            [z_half[1][0][1], z_half[1][1][0]],
            [z_half[1][1][1], z_half[0][0][0]],
        ]
        for s in range(2):
            for c in range(NCH):
                nc.tensor.matmul(
                    sc_banks[s][c][:1, :], cw_sb, h_sb[s][c], start=True, stop=True
                )
            for c in range(NCH):
                nc.vector.tensor_scalar_add(
                    out=out_sb[:, s, c * CHUNK : (c + 1) * CHUNK],
                    in0=sc_banks[s][c][:1, :],
                    scalar1=bb_sb,
                )
            nc.gpsimd.dma_start(
                out=out[s : s + 1, :].unsqueeze(0), in_=out_sb[:, s, :].unsqueeze(1)
            )


_MODULE_CACHE: list = []


def get_module() -> bass.Bass:
    if not _MODULE_CACHE:
        _MODULE_CACHE.append(_build_module())
    return _MODULE_CACHE[0]


def shard_inputs(inputs: dict) -> list[dict]:
    """Full inputs -> per-core input maps (row-block sharding of adjT)."""
    def tile_seq(s):
        s16 = np.asarray(s, np.float32)[0].astype(np.float16)  # [N, D]
        return np.ascontiguousarray(s16.reshape(MT, 128, D).transpose(1, 0, 2))

    seq1 = tile_seq(inputs["seq1"])
    seq2 = tile_seq(inputs["seq2"])
    adj16 = (np.asarray(inputs["adj"], np.float32)[0] * ADJ_SCALE).astype(np.float16)
    w = np.ascontiguousarray(np.asarray(inputs["W"], np.float32))
    wbt = np.ascontiguousarray(np.asarray(inputs["Wb"], np.float32).T)
    bvec = np.asarray(inputs["b"], np.float32).reshape(H, 1).copy()
    bbvec = np.asarray(inputs["bb"], np.float32).reshape(1, 1).copy()

    in_maps = []
    for k in range(NC):
        in_maps.append(
            {
                "adjt": np.ascontiguousarray(
                    adj16[k * CS : (k + 1) * CS, :].T.reshape(MT, 128, CS).transpose(1, 0, 2)
                ),
                "seq1": seq1,
                "seq2": seq2,
                "w": w,
                "wbt": wbt,
                "bvec": bvec,
                "bbvec": bbvec,
            }
        )
    return in_maps


def gather_output(core_outs: list[np.ndarray], cc_label: np.ndarray) -> np.ndarray:
    """Per-core [2, CS] score blocks -> full [1, 2N] output.

    Scatter through cc_label mirrors the reference's .at[flat].set: entry
    (community k, position j) is the score of node cc_label[k, j].
    """
    sc1 = np.concatenate([o[0] for o in core_outs]).astype(np.float32)
    sc2 = np.concatenate([o[1] for o in core_outs]).astype(np.float32)
    flat = np.asarray(cc_label).reshape(-1)
    ret1 = np.zeros(N, np.float32)
    ret2 = np.zeros(N, np.float32)
    ret1[flat] = sc1
    ret2[flat] = sc2
    return np.concatenate([ret1, ret2])[None, :]


def kernel(**inputs) -> np.ndarray:
    nc = get_module()
    in_maps = shard_inputs(inputs)
    res = run_bass_kernel_spmd(nc, in_maps, core_ids=list(range(NC)))
    core_outs = [res.results[k]["out"] for k in range(NC)]
    return gather_output(core_outs, inputs["cc_label"])


if __name__ == "__main__":
    nc = get_module()
    print("module built ok")



# revision 2
# speedup vs baseline: 1.0494x; 1.0494x over previous
"""DGI (Deep Graph Infomax) forward kernel for 8 TRN2 NeuronCores.

Problem (all shapes hardcoded):
  seq1, seq2: [1, 8192, 128] f32   node features
  adj:        [1, 8192, 8192] f32  dense adjacency
  cc_label:   [8, 1024] i32        community partition (arange layout)
  W: [128,128], b: [128], Wb: [128,128], bb: [] f32
  out:        [1, 16384] f32       = concat(ret1, ret2)

Math per GCN branch: h = relu(adj @ (seq @ W) + b). We reassociate to
(adj @ seq) @ W so the big contraction uses natural-layout seq tiles as
the stationary operand and a host-transposed adj block as the moving
operand; everything then lives in "transposed" space (features on
partitions), where the community mean is a free-axis reduction and the
bilinear scores are a 1-column matmul.

Sharding: core k owns nodes [1024k, 1024k+1024) == community k (cc_label
is arange). Each core reads its adjT column block (16 MB fp16), the full
seqs (4 MB fp16, replicated), computes its 1024 scores per branch. No
collectives.

Per-core device program (everything on the PE in fp16 so nothing runs at
the fp32 quarter rate; adj is pre-scaled by 256 on the host to sit in
fp16's normal range; the scale is undone for free in the relu
activation's `scale`):
  ZT[d, n]   = sum_m seq_s[m, d] * adjT[m, n]   (fp16, 32 accumulating
                                                 matmuls per psum bank,
                                                 split into two m-halves so
                                                 half 1's epilogue overlaps
                                                 half 2's stream)
  aggT[h, n] = sum_d W[d, h] * ZT[d, n]         (fp16 via a cast in the
                                                 PSUM->SBUF copy)
  hT         = relu(aggT/256 + b)   (fp16 out; + free-axis accum -> sums)
  c          = sigmoid(sum / 1024)               [128, 1] fp16
  cw         = Wb @ c     (lhsT = Wb^T from host) [128, 1] fp16
  sc_s[n]    = sum_h hT_s[h, n] * cw[h] + bb     [1, 1024] per branch

Schedule notes (from trace analysis of the 93 us baseline):
  - DMA: per-group [seq chunk][adj group] pairs alternate between the two
    HWDGE queues (sync + scalar) so tiles arrive in m order at the ~430
    GB/s aggregate rate and the first matmul's data lands ~3 us after the
    framework preamble instead of ~10.
  - PE: ~6 dummy matmuls on a memset scratch tile run during the initial
    DMA wait so the HAM throttle (cold 1.2 GHz -> warm 2.4 GHz, ~3.4 us
    activity window) is already warm when real data arrives.
  - Scalar: a dummy 1-element sigmoid is the first activation so the
    activation-table pass loads the sigmoid table (which also serves
    Relu/Copy) once at t~7 us instead of on the epilogue critical path.
  - All half-1 PSUM->SBUF copies go on vector so the scalar engine's
    queue of DMA triggers is never blocked behind compute.
"""

import numpy as np

import concourse.bass as bass
import concourse.tile as tile
from concourse import bacc, mybir
from concourse.bass_utils import run_bass_kernel_spmd

N = 8192          # nodes
D = 128           # input feature dim
H = 128           # hidden dim
NC = 8            # communities / cores
CS = N // NC      # community size (nodes per core)
MT = N // 128     # number of 128-row m-tiles (64)
CHUNK = 512       # matmul moving free dim (psum bank width in fp32)
NCH = CS // CHUNK # n-chunks per core (2)
HALF = MT // 2    # m-tiles per psum accumulation half (32)

F32 = mybir.dt.float32
F16 = mybir.dt.float16
ADJ_SCALE = 256.0  # keeps fp16(adj*scale) in the normal range; undone in the relu

# m-tile counts per DMA group; groups alternate sync/scalar queues.
GROUPS = [2, 2, 4, 4, 4, 6, 6, 6, 8, 8, 8, 6]
assert sum(GROUPS) == MT
N_DUMMY_MM = 6     # PE warmup matmuls during the initial DMA wait


def _build_module() -> bass.Bass:
    nc = bacc.Bacc()

    adjt = nc.declare_dram_parameter("adjt", [128, MT, CS], F16, isOutput=False)
    seqs = nc.declare_dram_parameter("seqs", [128, MT, 2, D], F16, isOutput=False)
    w = nc.declare_dram_parameter("w", [D, H], F16, isOutput=False)
    wbt = nc.declare_dram_parameter("wbt", [H, H], F16, isOutput=False)
    bvec = nc.declare_dram_parameter("bvec", [H, 1], F32, isOutput=False)
    bbvec = nc.declare_dram_parameter("bbvec", [1, 1], F32, isOutput=False)
    out = nc.declare_dram_parameter("out", [2, CS], F32, isOutput=True)

    with tile.TileContext(nc) as tc:
        _emit(tc, adjt, seqs, w, wbt, bvec, bbvec, out)
    nc.finalize()
    return nc


def _emit(tc, adjt, seqs, w, wbt, bvec, bbvec, out):
    nc = tc.nc
    with (
        tc.tile_pool(name="singles", bufs=1) as singles,
        tc.tile_pool(name="adj_sync", bufs=3) as adj_sync,
        tc.tile_pool(name="adj_scal", bufs=3) as adj_scal,
        tc.tile_pool(name="misc", bufs=1) as misc,
        tc.tile_pool(name="psum", bufs=1, space="PSUM") as psum,
    ):
        # --- warmup: PE HAM ramp + activation-table preload ------------
        scratch = singles.tile([128, CHUNK], F16)
        nc.vector.memset(scratch, 1.0)
        sig_dummy = misc.tile([1, 1], F32)
        nc.scalar.activation(
            out=sig_dummy,
            in_=scratch[:1, :1],
            func=mybir.ActivationFunctionType.Sigmoid,
        )

        # params on the gpsimd (SWDGE) queue so they never contend with the
        # adj/seq HWDGE streams.
        w_sb = singles.tile([D, H], F16)
        nc.gpsimd.dma_start(out=w_sb, in_=w[:])
        wbt_sb = singles.tile([H, H], F16)
        nc.gpsimd.dma_start(out=wbt_sb, in_=wbt[:])
        b_sb = singles.tile([H, 1], F32)
        nc.gpsimd.dma_start(out=b_sb, in_=bvec[:])
        bb_sb = singles.tile([1, 1], F32)
        nc.gpsimd.dma_start(out=bb_sb, in_=bbvec[:])

        seq_sb = singles.tile([128, MT, 2, D], F16)

        # Z accumulators split by m-half: first half banks 0-3, second 4-7.
        z_half = [
            [
                [psum.tile([128, CHUNK], F32, name=f"z_ps_{h}_{s}_{c}") for c in range(NCH)]
                for s in range(2)
            ]
            for h in range(2)
        ]
        zt_sb = [
            [
                [misc.tile([128, CHUNK], F16, name=f"zt_sb_{h}_{s}_{c}") for c in range(NCH)]
                for s in range(2)
            ]
            for h in range(2)
        ]
        h_sb = [
            [misc.tile([128, CHUNK], F16, name=f"h_sb_{s}_{c}") for c in range(NCH)]
            for s in range(2)
        ]
        csum = [misc.tile([H, 1], F32, name=f"csum_{c}") for c in range(NCH)]

        # PE warmup matmuls into a bank whose first real write (start=True)
        # is 30+ us away; they only read the memset scratch tile.
        for _ in range(N_DUMMY_MM):
            nc.tensor.matmul(
                z_half[1][1][1], scratch[:, :128], scratch, start=True, stop=True
            )

        def emit_half1_copies():
            # Mid-stream, off the critical path: all on vector so scalar's
            # DMA trigger queue stays unblocked.
            for s in range(2):
                for c in range(NCH):
                    nc.vector.tensor_copy(out=zt_sb[0][s][c], in_=z_half[0][s][c])

        def emit_half1_agg():
            # First-pass W-contraction into the (now free) first-half banks.
            for s in range(2):
                for c in range(NCH):
                    nc.tensor.matmul(
                        z_half[0][s][c], w_sb, zt_sb[0][s][c], start=True, stop=False
                    )

        # --- main stream: per-group [seq chunk][adj group] on alternating
        # HWDGE queues, then the group's matmuls -------------------------
        t0 = 0
        for gi, gn in enumerate(GROUPS):
            q = nc.sync if gi % 2 == 0 else nc.scalar
            pool = adj_sync if gi % 2 == 0 else adj_scal
            q.dma_start(out=seq_sb[:, t0 : t0 + gn, :, :], in_=seqs[:, t0 : t0 + gn, :, :])
            adj_sb = pool.tile([128, gn, CS], F16, name=f"adj_sb_{gi % 2}", tag=f"adj_{gi % 2}", bufs=3)
            q.dma_start(out=adj_sb, in_=adjt[:, t0 : t0 + gn, :])
            for u in range(gn):
                t = t0 + u
                hh = 0 if t < HALF else 1
                for s in range(2):
                    lhsT = seq_sb[:, t, s, :]
                    for c in range(NCH):
                        nc.tensor.matmul(
                            z_half[hh][s][c],
                            lhsT,
                            adj_sb[:, u, c * CHUNK : (c + 1) * CHUNK],
                            start=(t % HALF == 0),
                            stop=(t % HALF == HALF - 1),
                        )
            t0 += gn
            if t0 - gn < HALF <= t0:
                emit_half1_copies()
            if t0 - gn < HALF + 16 <= t0:
                emit_half1_agg()

        # --- epilogue --------------------------------------------------
        # Branch 0 drives the sigmoid -> cw chain; its half-2 copies run on
        # vector (c0) + scalar (c1) in parallel right after the last main
        # matmul, while branch 1's copies/Wagg fill the PE behind them.
        nc.vector.tensor_copy(out=zt_sb[1][0][0], in_=z_half[1][0][0])
        nc.scalar.activation(
            out=zt_sb[1][0][1],
            in_=z_half[1][0][1],
            func=mybir.ActivationFunctionType.Copy,
        )
        for c in range(NCH):
            nc.tensor.matmul(
                z_half[0][0][c], w_sb, zt_sb[1][0][c], start=False, stop=True
            )
            nc.scalar.activation(
                out=h_sb[0][c],
                in_=z_half[0][0][c],
                func=mybir.ActivationFunctionType.Relu,
                bias=b_sb,
                scale=1.0 / ADJ_SCALE,
                accum_out=csum[c],
            )
        for c in range(NCH):
            nc.vector.tensor_copy(out=zt_sb[1][1][c], in_=z_half[1][1][c])

        csum_tot = misc.tile([H, 1], F32)
        nc.vector.tensor_add(out=csum_tot, in0=csum[0], in1=csum[1])
        c_sb = misc.tile([H, 1], F16)
        nc.scalar.activation(
            out=c_sb,
            in_=csum_tot,
            func=mybir.ActivationFunctionType.Sigmoid,
            scale=1.0 / CS,
        )

        for c in range(NCH):
            nc.tensor.matmul(
                z_half[0][1][c], w_sb, zt_sb[1][1][c], start=False, stop=True
            )
        cw_ps = z_half[1][0][0]
        nc.tensor.matmul(cw_ps[:, :1], wbt_sb, c_sb, start=True, stop=True)
        for c in range(NCH):
            nc.scalar.activation(
                out=h_sb[1][c],
                in_=z_half[0][1][c],
                func=mybir.ActivationFunctionType.Relu,
                bias=b_sb,
                scale=1.0 / ADJ_SCALE,
            )
        cw_sb = misc.tile([H, 1], F16)
        nc.vector.tensor_copy(out=cw_sb, in_=cw_ps[:, :1])

        out_sb = misc.tile([1, 2, CS], F32)
        sc_banks = [
            [z_half[1][0][1], z_half[1][1][0]],
            [z_half[1][1][1], z_half[0][0][0]],
        ]
        for s in range(2):
            for c in range(NCH):
                nc.tensor.matmul(
                    sc_banks[s][c][:1, :], cw_sb, h_sb[s][c], start=True, stop=True
                )
            for c in range(NCH):
                nc.vector.tensor_scalar_add(
                    out=out_sb[:, s, c * CHUNK : (c + 1) * CHUNK],
                    in0=sc_banks[s][c][:1, :],
                    scalar1=bb_sb,
                )
        # single HWDGE store of both branches' scores
        nc.scalar.dma_start(out=out[:, :].unsqueeze(0), in_=out_sb)


_MODULE_CACHE: list = []


def get_module() -> bass.Bass:
    if not _MODULE_CACHE:
        _MODULE_CACHE.append(_build_module())
    return _MODULE_CACHE[0]


def shard_inputs(inputs: dict) -> list[dict]:
    """Full inputs -> per-core input maps (row-block sharding of adjT)."""
    def tile_seq(s):
        s16 = np.asarray(s, np.float32)[0].astype(np.float16)  # [N, D]
        return s16.reshape(MT, 128, D).transpose(1, 0, 2)

    # interleave both branches so one DMA chunk feeds both: [128, MT, 2, D]
    seqs = np.ascontiguousarray(
        np.stack([tile_seq(inputs["seq1"]), tile_seq(inputs["seq2"])], axis=2)
    )
    adj16 = (np.asarray(inputs["adj"], np.float32)[0] * ADJ_SCALE).astype(np.float16)
    w = np.ascontiguousarray(np.asarray(inputs["W"], np.float32).astype(np.float16))
    wbt = np.ascontiguousarray(np.asarray(inputs["Wb"], np.float32).T.astype(np.float16))
    bvec = np.asarray(inputs["b"], np.float32).reshape(H, 1).copy()
    bbvec = np.asarray(inputs["bb"], np.float32).reshape(1, 1).copy()

    in_maps = []
    for k in range(NC):
        in_maps.append(
            {
                "adjt": np.ascontiguousarray(
                    adj16[k * CS : (k + 1) * CS, :].T.reshape(MT, 128, CS).transpose(1, 0, 2)
                ),
                "seqs": seqs,
                "w": w,
                "wbt": wbt,
                "bvec": bvec,
                "bbvec": bbvec,
            }
        )
    return in_maps


def gather_output(core_outs: list[np.ndarray], cc_label: np.ndarray) -> np.ndarray:
    """Per-core [2, CS] score blocks -> full [1, 2N] output.

    Scatter through cc_label mirrors the reference's .at[flat].set: entry
    (community k, position j) is the score of node cc_label[k, j].
    """
    sc1 = np.concatenate([o[0] for o in core_outs]).astype(np.float32)
    sc2 = np.concatenate([o[1] for o in core_outs]).astype(np.float32)
    flat = np.asarray(cc_label).reshape(-1)
    ret1 = np.zeros(N, np.float32)
    ret2 = np.zeros(N, np.float32)
    ret1[flat] = sc1
    ret2[flat] = sc2
    return np.concatenate([ret1, ret2])[None, :]


def kernel(**inputs) -> np.ndarray:
    nc = get_module()
    in_maps = shard_inputs(inputs)
    res = run_bass_kernel_spmd(nc, in_maps, core_ids=list(range(NC)))
    core_outs = [res.results[k]["out"] for k in range(NC)]
    return gather_output(core_outs, inputs["cc_label"])


if __name__ == "__main__":
    nc = get_module()
    print("module built ok")
